# revision 77
# baseline (speedup 1.0000x reference)
"""Multi-head self-attention (B=2, S=2048, D=1024, H=16, causal) on 8 trn2 cores.

Sharding: core c computes heads {2c, 2c+1} for both batches (column-parallel
QKV, row-parallel O). Each core returns a partial [4096, 1024] output; the
host sums the 8 partials.

v1 design (vs the f32r baseline):
  - all projection inputs bf16 (x, Wq, Wk, Wv): halves the DMA volume and
    keeps matmuls at 1 cycle/row; Wo/avt stay f32r (same speed, better
    accuracy).
  - batched DMAs: 8 startup issues instead of 23 (DIRECT2D issue on the
    sync sequencer costs ~0.6us each and gates the first matmul).
  - V is projected directly into [token, dim] layout (lhsT = x tile), so
    no PE transposes, no identity const, no vt intermediate.
  - batch 1's QKV runs as PE filler units inside batch 0's attention
    j-loops so the tensor engine never idles (p-state stays at 2.4 GHz).
  - scores for both heads land in one [128,2,512] PSUM tile -> a single
    exp per j (halves ACT instruction overhead).
  - softmax normalization r = 1/denom via vector.reciprocal on DVE plus a
    single f32r broadcast matmul: the scalar engine runs exp only, so the
    1283ns ACT table re-loads (exp<->ln in the baseline) disappear.
  - batch 0's normalize + O-projection and batch 1's O-projection spread
    as fillers through batch 1's attention; only (1,3) remains as tail.
"""

import os
import numpy as np
from contextlib import ExitStack

import ml_dtypes

import concourse.bass as bass
import concourse.tile as tile
from concourse import bacc, mybir
from concourse.bass_utils import run_bass_kernel_spmd

F32R = mybir.dt.float32r
F32 = mybir.dt.float32
BF16 = mybir.dt.bfloat16
EXP = mybir.ActivationFunctionType.Exp

B, S, D = 2, 2048, 1024
NT = B * S            # 4096 tokens total
NCORES = 8
SCALE = 0.125         # 1/sqrt(64)
BFNP = ml_dtypes.bfloat16

_BUILT = None
LAST_RESULTS = None


def _build():
    nc = bacc.Bacc("TRN2", target_bir_lowering=False, debug=False,
                   num_devices=NCORES)
    # xt[p, k, t] = x[t, 128k + p]
    xt_d = nc.dram_tensor("xt", [128, 8, NT], BF16, kind="ExternalInput").ap()
    wq_d = nc.dram_tensor("wq", [128, D], BF16, kind="ExternalInput").ap()
    # cb = wk | wvT | tri  packed to load in one DMA
    cb_d = nc.dram_tensor("cb", [128, 2176], BF16, kind="ExternalInput").ap()
    wo_d = nc.dram_tensor("wo", [128, 1024], BF16, kind="ExternalInput").ap()
    # row 64: cols 0:64 ones (head-0 lhsT), cols 192:256 ones (head-1 lhsT)
    onesb_d = nc.dram_tensor("onesb", [65, 256], F32R,
                             kind="ExternalInput").ap()
    out_d = nc.dram_tensor("out", [NT, D], BF16, kind="ExternalOutput").ap()

    with tile.TileContext(nc) as tc, ExitStack() as ctx:
        consts = ctx.enter_context(tc.tile_pool(name="consts", bufs=1))
        sb = ctx.enter_context(tc.tile_pool(name="sb", bufs=1))
        ps = ctx.enter_context(tc.tile_pool(name="ps", bufs=1, space="PSUM"))

        # ---- tiles (allocation only; DMAs emitted in priority order) ----
        wq_t = consts.tile([128, D], BF16, tag="wq")
        cb_t = consts.tile([128, 2176], BF16, tag="cb")
        wo_t = consts.tile([128, 1024], BF16, tag="wo")
        onesb_t = consts.tile([65, 256], F32R, tag="onesb")
        kt_w = cb_t[:, 0:1024]
        wvT = cb_t[:, 1024:2048]
        tri_t = cb_t[:, 2048:2176]

        # xt chunk tiles; (0,0) is split into two k-half tiles so the very
        # first matmuls wait only on the first half's DMA
        xt_tiles = {}
        for b in range(B):
            ent = []
            for c in range(4):
                if b == 0 and c == 0:
                    ta = sb.tile([128, 4, 512], BF16, tag="xth", bufs=2,
                                 name="xt0_0a")
                    tb = sb.tile([128, 4, 512], BF16, tag="xth", bufs=2,
                                 name="xt0_0b")
                    ent.append([(0, 4, ta), (4, 4, tb)])
                else:
                    t = sb.tile([128, 8, 512], BF16, tag="xt", bufs=4,
                                name=f"xt{b}_{c}")
                    ent.append([(0, 8, t)])
            xt_tiles[b] = ent

        def xt_ap(b, c, k, cols=slice(0, 512)):
            for k0, nk, t in reversed(xt_tiles[b][c]):
                if k >= k0:
                    return t[:, k - k0, cols]

        def dma_xt(b, c):
            for k0, nk, t in xt_tiles[b][c]:
                nc.sync.dma_start(
                    t, xt_d[:, k0:k0 + nk,
                            S * b + 512 * c:S * b + 512 * (c + 1)])

        # startup DMA order: first-needed first (sync queue is FIFO)
        nc.sync.dma_start(wq_t, wq_d)
        dma_xt(0, 0)
        nc.sync.dma_start(cb_t, cb_d)
        dma_xt(0, 1)
        dma_xt(0, 2)
        nc.sync.dma_start(wo_t, wo_d)
        nc.sync.dma_start(onesb_t, onesb_d)
        dma_xt(0, 3)

        # ---- projections as callable units (so b1 can fill b0's attn) ----
        proj_out = {}

        def proj_units(b):
            qt = sb.tile([128, S], BF16, tag="qt", bufs=2, name=f"qt{b}")
            # K stored twice, zero-padded per head to contraction depth 128:
            # K=64 matmuls run at ~1.5 cycles/row on HW, K=128 at 1.0, and
            # the zero rows annihilate the other head's Q rows.
            ktz = [sb.tile([128, S], BF16, tag=f"ktz{h}", bufs=2,
                           name=f"ktz{b}_{h}") for h in range(2)]
            vg = sb.tile([128, 16, 130], BF16, tag="vg", bufs=2,
                         name=f"vg{b}")
            proj_out[b] = (qt, ktz, vg)
            units = []

            def ones_unit():
                nc.gpsimd.memset(vg[:, :, 64:65], 1.0)
                nc.gpsimd.memset(vg[:, :, 129:130], 1.0)
                nc.gpsimd.memset(ktz[0][64:128, :], 0.0)
                nc.gpsimd.memset(ktz[1][0:64, :], 0.0)

            units.append(ones_unit)

            def qk_unit(c, w_t, dst):
                pp = ps.tile([128, 512], F32, tag="op", bufs=2,
                             name=f"pp{b}_{c}")
                for k in range(8):
                    nc.tensor.matmul(pp, lhsT=w_t[:, 128 * k:128 * (k + 1)],
                                     rhs=xt_ap(b, c, k),
                                     start=(k == 0), stop=(k == 7))
                if dst is None:
                    cs = slice(512 * c, 512 * (c + 1))
                    nc.vector.tensor_copy(ktz[0][0:64, cs], pp[0:64, :])
                    nc.vector.tensor_copy(ktz[1][64:128, cs], pp[64:128, :])
                else:
                    nc.vector.tensor_copy(dst[:, 512 * c:512 * (c + 1)], pp)

            def v_unit(c, lo, hi, pv):
                # token-blocks tb4 in [lo,hi) of chunk c; pv packs 4 blocks
                for tb4 in range(lo, hi):
                    cols = slice(128 * tb4, 128 * (tb4 + 1))
                    for k in range(8):
                        nc.tensor.matmul(
                            pv[:, 128 * tb4:128 * (tb4 + 1)],
                            lhsT=xt_ap(b, c, k, cols),
                            rhs=wvT[:, 128 * k:128 * (k + 1)],
                            start=(k == 0), stop=(k == 7),
                            skip_group_check=True)
                    j = 4 * c + tb4
                    nc.vector.tensor_copy(
                        vg[:, j, 0:64], pv[:, 128 * tb4:128 * tb4 + 64])
                    nc.vector.tensor_copy(
                        vg[:, j, 65:129], pv[:, 128 * tb4 + 64:128 * (tb4 + 1)])

            for c in range(4):
                units.append(lambda c=c: qk_unit(c, wq_t, qt))
                units.append(lambda c=c: qk_unit(c, kt_w, None))
                pv = ps.tile([128, 512], F32, tag="op", bufs=2,
                             name=f"pv{b}_{c}")
                units.append(lambda c=c, pv=pv: v_unit(c, 0, 2, pv))
                units.append(lambda c=c, pv=pv: v_unit(c, 2, 4, pv))
            return units

        # ---- deferred normalize (part1) and O projection (part2) ----
        avt_of = {}

        def part1(b, qc, rawf):
            cs = slice(512 * qc, 512 * (qc + 1))
            avt = sb.tile([128, 512], BF16, tag="avt", bufs=4,
                          name=f"avt{b}_{qc}")
            rawsh = sb.tile([128, 512], F32R, tag="rawsh", bufs=2,
                            name=f"rawsh{b}_{qc}")
            recb = sb.tile([128, 512], F32, tag="recb", bufs=2,
                           name=f"recb{b}_{qc}")
            nc.sync.dma_start(rawsh[64:128, :], rawf[1][0:64, cs])
            # broadcast both heads' denominator rows across partitions with
            # two K=1 matmuls (no DMA round-trip on the critical chain)
            bc = ps.tile([128, 512], F32, tag="op", bufs=2,
                         name=f"bc{b}_{qc}")
            nc.tensor.matmul(bc, lhsT=onesb_t[64:65, 0:128],
                             rhs=rawf[0][64:65, cs], start=True, stop=False,
                             skip_group_check=True)
            nc.tensor.matmul(bc, lhsT=onesb_t[64:65, 128:256],
                             rhs=rawf[1][64:65, cs], start=False, stop=True,
                             skip_group_check=True)
            # ~18-bit reciprocal, ~5x faster than the exact DVE reciprocal
            # (denominators are sums of positive exps, so no edge cases)
            nc.vector.reciprocal_approx_fast(out=recb, in_=bc)
            nc.vector.tensor_mul(avt[0:64, :], rawf[0][0:64, cs],
                                 recb[0:64, :])
            nc.vector.tensor_mul(avt[64:128, :], rawsh[64:128, :],
                                 recb[64:128, :])
            avt_of[(b, qc)] = avt

        COPY = mybir.ActivationFunctionType.Copy

        def part2_unit(b, qc, tt, pstag="op", split_dma=False):
            # chv0 copy on DVE, chv1 on ACT (Copy is in every act table, so
            # it never triggers a table re-load); tail units borrow the then
            # idle "sc" PSUM buffers to avoid WAR serialization
            avt = avt_of[(b, qc)]
            ost = sb.tile([128, 1024], BF16, tag="ost", bufs=6,
                          name=f"ost{b}_{qc}_{tt}")
            if pstag == "sc":
                big = ps.tile([128, 2, 512], F32, tag="sc", bufs=2,
                              name=f"opb{b}_{qc}_{tt}")
                ops = [big[:, 0, :], big[:, 1, :]]
            else:
                ops = [ps.tile([128, 512], F32, tag="op", bufs=2,
                               name=f"op{b}_{qc}_{tt}_{chv}")
                       for chv in range(2)]
            row0 = S * b + 512 * qc + 128 * tt
            for chv in range(2):
                nc.tensor.matmul(
                    ops[chv], lhsT=avt[:, 128 * tt:128 * (tt + 1)],
                    rhs=wo_t[:, 512 * chv:512 * (chv + 1)],
                    start=True, stop=True)
                dst = ost[:, 512 * chv:512 * (chv + 1)]
                if chv == 0:
                    nc.vector.tensor_copy(dst, ops[chv])
                else:
                    nc.scalar.activation(dst, ops[chv], COPY)
                if split_dma:
                    nc.sync.dma_start(
                        out_d[row0:row0 + 128, 512 * chv:512 * (chv + 1)],
                        dst)
            if not split_dma:
                nc.sync.dma_start(out_d[row0:row0 + 128, :], ost)

        # ---- attention ----
        rawf_of = {}

        def attention(b, p1_at, fillers):
            qt, ktz, vg = proj_out[b]
            rawf = [sb.tile([65, S], F32R, tag=f"rawf{h}", bufs=2,
                            name=f"rawf{b}_{h}") for h in range(2)]
            rawf_of[b] = rawf
            for qc in range(4):
                njt = 4 * qc + 4
                avps = [ps.tile([65, 512], F32, tag="av", bufs=2,
                                name=f"avps{b}_{qc}_{h}") for h in range(2)]
                fl = fillers[qc]
                nfl = len(fl)
                pend = []

                def do_av(j, et, avps=avps, qc=qc, njt=njt, vg=vg):
                    vs = max(0, 128 * (j - 4 * qc))
                    for h in range(2):
                        nc.tensor.matmul(
                            avps[h][0:65, vs:512],
                            lhsT=vg[:, j, 65 * h:65 * h + 65],
                            rhs=et[:, h, vs:512],
                            start=(j == 0), stop=(j == njt - 1),
                            skip_group_check=True)

                for j in range(njt):
                    vs = max(0, 128 * (j - 4 * qc))
                    sc = ps.tile([128, 2, 512], F32, tag="sc", bufs=2,
                                 name=f"sc{b}_{qc}_{j}")
                    for h in range(2):
                        nc.tensor.matmul(
                            sc[:, h, vs:512],
                            lhsT=ktz[h][:, 128 * j:128 * (j + 1)],
                            rhs=qt[:, 512 * qc + vs:512 * (qc + 1)],
                            start=True, stop=True)
                    et = sb.tile([128, 2, 512], BF16, tag="et", bufs=4,
                                 name=f"et{b}_{qc}_{j}")
                    nc.scalar.activation(et[:, :, vs:512], sc[:, :, vs:512],
                                         EXP, scale=SCALE)
                    if j >= 4 * qc:
                        for h in range(2):
                            nc.vector.tensor_mul(et[:, h, vs:vs + 128],
                                                 et[:, h, vs:vs + 128],
                                                 tri_t)
                    pend.append((j, et))
                    if len(pend) > 2:   # lag-2 software pipeline
                        do_av(*pend.pop(0))
                    if j == 1 and p1_at[qc] is not None:
                        p1_at[qc]()
                    # spread fillers over all but the last iteration: the
                    # rawf copies below gate the next qc's AV chain, so they
                    # must not queue behind the last slot's filler work
                    if j < njt - 1:
                        k0 = nfl * j // njt
                        k1 = nfl * (j + 1) // njt
                        for k in range(k0, k1):
                            fl[k]()
                for args in pend:
                    do_av(*args)
                for h in range(2):
                    nc.vector.tensor_copy(rawf[h][:, 512 * qc:512 * (qc + 1)],
                                          avps[h][0:65, :])
                for k in range(nfl * (njt - 1) // njt, nfl):
                    fl[k]()

        # ================= main flow =================
        # p-state warm-up: the PE needs ~3us of continuous work to reach
        # 2.4 GHz (else 1.2), and it would otherwise idle ~4.5us waiting for
        # the first DMAs. A stream of zero-input dummy matmuls (no data
        # deps) ramps the clock inside that dead window so the real
        # projections start at full speed.
        zt = sb.tile([128, 256], BF16, tag="zt", bufs=1, name="zt")
        nc.gpsimd.memset(zt, 0.0)
        dps = ps.tile([128, 256], F32, tag="op", bufs=2, name="dummy_ps")
        for _ in range(22):
            nc.tensor.matmul(dps, lhsT=zt[:, 0:128], rhs=zt,
                             start=True, stop=True, skip_group_check=True)

        units0 = proj_units(0)
        for u in units0:
            u()

        # b1 inputs: xt DMAs issue now (sync sequencer is idle during b0
        # attention; WAR on the xt pool delays them behind b0's V reads)
        for c in range(4):
            dma_xt(1, c)
        units1 = proj_units(1)

        def mk_p1(b, qc):
            return lambda: part1(b, qc, rawf_of[b])

        def mk_u(b, qc, tt):
            return lambda: part2_unit(b, qc, tt)

        # b0 attention: b1's projections + b0's own (per-qc-ready)
        # normalize/O-projection as PE fillers
        b0f0 = units1[0:4]                      # ones, Qc0, Kc0, Vc0a
        b0f1 = (units1[4:9] +                   # Vc0b, Qc1, Kc1, Vc1a/b
                [mk_u(0, 0, 0), mk_u(0, 0, 1), mk_u(0, 0, 2)])
        b0f2 = (units1[9:13] +                  # Qc2, Kc2, Vc2a/b
                [mk_u(0, 0, 3),
                 mk_u(0, 1, 0), mk_u(0, 1, 1), mk_u(0, 1, 2),
                 mk_u(0, 1, 3)])
        b0f3 = (units1[13:17] +                 # Qc3, Kc3, Vc3a/b
                [mk_u(0, 2, 0), mk_u(0, 2, 1), mk_u(0, 2, 2),
                 mk_u(0, 2, 3)])
        attention(0, p1_at=[None, mk_p1(0, 0), mk_p1(0, 1), mk_p1(0, 2)],
                  fillers=[b0f0, b0f1, b0f2, b0f3])

        # b1 attention: leftover b0 units + b1's own normalize/O
        f0 = [mk_u(0, 3, 0), mk_u(0, 3, 1), mk_u(0, 3, 2)]
        f1 = [mk_u(0, 3, 3),
              mk_u(1, 0, 0), mk_u(1, 0, 1), mk_u(1, 0, 2), mk_u(1, 0, 3)]
        f2 = [mk_u(1, 1, 0), mk_u(1, 1, 1)]
        f3 = [mk_u(1, 2, 0)]
        attention(1, p1_at=[mk_p1(0, 3), mk_p1(1, 0), mk_p1(1, 1),
                            mk_p1(1, 2)],
                  fillers=[f0, f1, f2, f3])

        # tail: the reserved units go FIRST so the in-order PE queue
        # executes them while part1(1,3)'s DVE chain resolves
        part2_unit(1, 1, 2, pstag="sc")
        part2_unit(1, 1, 3, pstag="op")
        part2_unit(1, 2, 1, pstag="sc")
        part2_unit(1, 2, 2, pstag="op")
        part2_unit(1, 2, 3, pstag="sc")
        part1(1, 3, rawf_of[1])
        for tt in range(4):
            part2_unit(1, 3, tt, pstag="op" if tt % 2 == 0 else "sc",
                       split_dma=True)
    nc.compile()
    return nc


def _get_built():
    global _BUILT
    if _BUILT is None:
        _BUILT = _build()
    return _BUILT


def _wslice(w, c):
    # [p, 8k x 128m]: w_sb[p, 128k+m] = w[128c+m, 128k+p]
    a = w[128 * c:128 * (c + 1)].reshape(128, 8, 128)
    return np.ascontiguousarray(a.transpose(2, 1, 0).reshape(128, D))


def _host_inputs(x, q_proj, k_proj, v_proj, o_proj):
    xr = x.reshape(NT, 8, 128).transpose(2, 1, 0)   # [p, k, t]
    xt = np.ascontiguousarray(xr).astype(BFNP)
    tri = np.triu(np.ones((128, 128), dtype=np.float32)).astype(BFNP)
    onesb = np.zeros((65, 256), dtype=np.float32)
    onesb[64, 0:64] = 1.0
    onesb[64, 192:256] = 1.0

    in_maps = []
    for c in range(NCORES):
        wq = _wslice(q_proj, c).astype(BFNP)
        wk = _wslice(k_proj, c).astype(BFNP)
        wvT = _wslice(v_proj, c).astype(BFNP)
        cb = np.ascontiguousarray(np.concatenate([wk, wvT, tri], axis=1))
        wo = np.ascontiguousarray(
            o_proj[:, 128 * c:128 * (c + 1)].T).astype(BFNP)
        in_maps.append(dict(xt=xt, wq=wq, cb=cb, wo=wo, onesb=onesb))
    return in_maps


def kernel(**inputs):
    x = np.asarray(inputs["x"], dtype=np.float32)
    q_proj = np.asarray(inputs["q_proj"], dtype=np.float32)
    k_proj = np.asarray(inputs["k_proj"], dtype=np.float32)
    v_proj = np.asarray(inputs["v_proj"], dtype=np.float32)
    o_proj = np.asarray(inputs["o_proj"], dtype=np.float32)

    in_maps = _host_inputs(x, q_proj, k_proj, v_proj, o_proj)
    nc = _get_built()
    global LAST_RESULTS
    LAST_RESULTS = run_bass_kernel_spmd(
        nc, in_maps, core_ids=list(range(NCORES)),
        trace=bool(os.environ.get("KERNEL_TRACE")))
    acc = np.asarray(LAST_RESULTS.results[0]["out"]).astype(np.float32)
    for c in range(1, NCORES):
        acc += np.asarray(LAST_RESULTS.results[c]["out"]).astype(np.float32)
    return acc.reshape(B, S, D)


# revision 79
# speedup vs baseline: 1.0104x; 1.0104x over previous
"""Multi-head self-attention (B=2, S=2048, D=1024, H=16, causal) on 8 trn2 cores.

Sharding: core c computes heads {2c, 2c+1} for both batches (column-parallel
QKV, row-parallel O). Each core returns a partial [4096, 1024] output; the
host sums the 8 partials.

v1 design (vs the f32r baseline):
  - all projection inputs bf16 (x, Wq, Wk, Wv): halves the DMA volume and
    keeps matmuls at 1 cycle/row; Wo/avt stay f32r (same speed, better
    accuracy).
  - batched DMAs: 8 startup issues instead of 23 (DIRECT2D issue on the
    sync sequencer costs ~0.6us each and gates the first matmul).
  - V is projected directly into [token, dim] layout (lhsT = x tile), so
    no PE transposes, no identity const, no vt intermediate.
  - batch 1's QKV runs as PE filler units inside batch 0's attention
    j-loops so the tensor engine never idles (p-state stays at 2.4 GHz).
  - scores for both heads land in one [128,2,512] PSUM tile -> a single
    exp per j (halves ACT instruction overhead).
  - softmax normalization r = 1/denom via vector.reciprocal on DVE plus a
    single f32r broadcast matmul: the scalar engine runs exp only, so the
    1283ns ACT table re-loads (exp<->ln in the baseline) disappear.
  - batch 0's normalize + O-projection and batch 1's O-projection spread
    as fillers through batch 1's attention; only (1,3) remains as tail.
"""

import os
import numpy as np
from contextlib import ExitStack

import ml_dtypes

import concourse.bass as bass
import concourse.tile as tile
from concourse import bacc, mybir
from concourse.bass_utils import run_bass_kernel_spmd

F32R = mybir.dt.float32r
F32 = mybir.dt.float32
BF16 = mybir.dt.bfloat16
EXP = mybir.ActivationFunctionType.Exp

B, S, D = 2, 2048, 1024
NT = B * S            # 4096 tokens total
NCORES = 8
SCALE = 0.125         # 1/sqrt(64)
BFNP = ml_dtypes.bfloat16

_BUILT = None
LAST_RESULTS = None


def _build():
    nc = bacc.Bacc("TRN2", target_bir_lowering=False, debug=False,
                   num_devices=NCORES)
    # xt[p, k, t] = x[t, 128k + p]
    xt_d = nc.dram_tensor("xt", [128, 8, NT], BF16, kind="ExternalInput").ap()
    wq_d = nc.dram_tensor("wq", [128, D], BF16, kind="ExternalInput").ap()
    # cb = wk | wvT | tri  packed to load in one DMA
    cb_d = nc.dram_tensor("cb", [128, 2176], BF16, kind="ExternalInput").ap()
    wo_d = nc.dram_tensor("wo", [128, 1024], BF16, kind="ExternalInput").ap()
    # row 64: cols 0:64 ones (head-0 lhsT), cols 192:256 ones (head-1 lhsT)
    onesb_d = nc.dram_tensor("onesb", [65, 256], F32R,
                             kind="ExternalInput").ap()
    out_d = nc.dram_tensor("out", [NT, D], BF16, kind="ExternalOutput").ap()

    with tile.TileContext(nc) as tc, ExitStack() as ctx:
        consts = ctx.enter_context(tc.tile_pool(name="consts", bufs=1))
        sb = ctx.enter_context(tc.tile_pool(name="sb", bufs=1))
        ps = ctx.enter_context(tc.tile_pool(name="ps", bufs=1, space="PSUM"))

        # ---- tiles (allocation only; DMAs emitted in priority order) ----
        wq_t = consts.tile([128, D], BF16, tag="wq")
        cb_t = consts.tile([128, 2176], BF16, tag="cb")
        wo_t = consts.tile([128, 1024], BF16, tag="wo")
        onesb_t = consts.tile([65, 256], F32R, tag="onesb")
        kt_w = cb_t[:, 0:1024]
        wvT = cb_t[:, 1024:2048]
        tri_t = cb_t[:, 2048:2176]

        # xt chunk tiles; (0,0) is split into two k-half tiles so the very
        # first matmuls wait only on the first half's DMA
        xt_tiles = {}
        for b in range(B):
            ent = []
            for c in range(4):
                if b == 0 and c == 0:
                    ta = sb.tile([128, 4, 512], BF16, tag="xth", bufs=2,
                                 name="xt0_0a")
                    tb = sb.tile([128, 4, 512], BF16, tag="xth", bufs=2,
                                 name="xt0_0b")
                    ent.append([(0, 4, ta), (4, 4, tb)])
                else:
                    t = sb.tile([128, 8, 512], BF16, tag="xt", bufs=4,
                                name=f"xt{b}_{c}")
                    ent.append([(0, 8, t)])
            xt_tiles[b] = ent

        def xt_ap(b, c, k, cols=slice(0, 512)):
            for k0, nk, t in reversed(xt_tiles[b][c]):
                if k >= k0:
                    return t[:, k - k0, cols]

        def dma_xt(b, c):
            for k0, nk, t in xt_tiles[b][c]:
                nc.sync.dma_start(
                    t, xt_d[:, k0:k0 + nk,
                            S * b + 512 * c:S * b + 512 * (c + 1)])

        # startup DMA order: first-needed first (sync queue is FIFO)
        nc.sync.dma_start(wq_t, wq_d)
        dma_xt(0, 0)
        nc.sync.dma_start(cb_t, cb_d)
        dma_xt(0, 1)
        dma_xt(0, 2)
        nc.sync.dma_start(wo_t, wo_d)
        nc.sync.dma_start(onesb_t, onesb_d)
        dma_xt(0, 3)

        # ---- projections as callable units (so b1 can fill b0's attn) ----
        proj_out = {}

        def proj_units(b):
            qt = sb.tile([128, S], BF16, tag="qt", bufs=2, name=f"qt{b}")
            # K stored twice, zero-padded per head to contraction depth 128:
            # K=64 matmuls run at ~1.5 cycles/row on HW, K=128 at 1.0, and
            # the zero rows annihilate the other head's Q rows.
            ktz = [sb.tile([128, S], BF16, tag=f"ktz{h}", bufs=2,
                           name=f"ktz{b}_{h}") for h in range(2)]
            vg = sb.tile([128, 16, 130], BF16, tag="vg", bufs=2,
                         name=f"vg{b}")
            proj_out[b] = (qt, ktz, vg)
            units = []

            def ones_unit():
                nc.gpsimd.memset(vg[:, :, 64:65], 1.0)
                nc.gpsimd.memset(vg[:, :, 129:130], 1.0)
                nc.gpsimd.memset(ktz[0][64:128, :], 0.0)
                nc.gpsimd.memset(ktz[1][0:64, :], 0.0)

            units.append(ones_unit)

            def qk_unit(c, w_t, dst):
                pp = ps.tile([128, 512], F32, tag="op", bufs=2,
                             name=f"pp{b}_{c}")
                for k in range(8):
                    nc.tensor.matmul(pp, lhsT=w_t[:, 128 * k:128 * (k + 1)],
                                     rhs=xt_ap(b, c, k),
                                     start=(k == 0), stop=(k == 7))
                if dst is None:
                    cs = slice(512 * c, 512 * (c + 1))
                    nc.vector.tensor_copy(ktz[0][0:64, cs], pp[0:64, :])
                    nc.vector.tensor_copy(ktz[1][64:128, cs], pp[64:128, :])
                else:
                    nc.vector.tensor_copy(dst[:, 512 * c:512 * (c + 1)], pp)

            def v_unit(c, lo, hi, pv):
                # token-blocks tb4 in [lo,hi) of chunk c; pv packs 4 blocks
                for tb4 in range(lo, hi):
                    cols = slice(128 * tb4, 128 * (tb4 + 1))
                    for k in range(8):
                        nc.tensor.matmul(
                            pv[:, 128 * tb4:128 * (tb4 + 1)],
                            lhsT=xt_ap(b, c, k, cols),
                            rhs=wvT[:, 128 * k:128 * (k + 1)],
                            start=(k == 0), stop=(k == 7),
                            skip_group_check=True)
                    j = 4 * c + tb4
                    nc.vector.tensor_copy(
                        vg[:, j, 0:64], pv[:, 128 * tb4:128 * tb4 + 64])
                    nc.vector.tensor_copy(
                        vg[:, j, 65:129], pv[:, 128 * tb4 + 64:128 * (tb4 + 1)])

            for c in range(4):
                units.append(lambda c=c: qk_unit(c, wq_t, qt))
                units.append(lambda c=c: qk_unit(c, kt_w, None))
                pv = ps.tile([128, 512], F32, tag="op", bufs=2,
                             name=f"pv{b}_{c}")
                units.append(lambda c=c, pv=pv: v_unit(c, 0, 2, pv))
                units.append(lambda c=c, pv=pv: v_unit(c, 2, 4, pv))
            return units

        # ---- deferred normalize (part1) and O projection (part2) ----
        avt_of = {}

        def part1(b, qc, rawf):
            cs = slice(512 * qc, 512 * (qc + 1))
            avt = sb.tile([128, 512], BF16, tag="avt", bufs=4,
                          name=f"avt{b}_{qc}")
            rawsh = sb.tile([128, 512], F32R, tag="rawsh", bufs=2,
                            name=f"rawsh{b}_{qc}")
            recb = sb.tile([128, 512], F32, tag="recb", bufs=2,
                           name=f"recb{b}_{qc}")
            nc.sync.dma_start(rawsh[64:128, :], rawf[1][0:64, cs])
            # broadcast both heads' denominator rows across partitions with
            # two K=1 matmuls (no DMA round-trip on the critical chain)
            bc = ps.tile([128, 512], F32, tag="op", bufs=2,
                         name=f"bc{b}_{qc}")
            nc.tensor.matmul(bc, lhsT=onesb_t[64:65, 0:128],
                             rhs=rawf[0][64:65, cs], start=True, stop=False,
                             skip_group_check=True)
            nc.tensor.matmul(bc, lhsT=onesb_t[64:65, 128:256],
                             rhs=rawf[1][64:65, cs], start=False, stop=True,
                             skip_group_check=True)
            # ~18-bit reciprocal, ~5x faster than the exact DVE reciprocal
            # (denominators are sums of positive exps, so no edge cases)
            nc.vector.reciprocal_approx_fast(out=recb, in_=bc)
            nc.vector.tensor_mul(avt[0:64, :], rawf[0][0:64, cs],
                                 recb[0:64, :])
            nc.vector.tensor_mul(avt[64:128, :], rawsh[64:128, :],
                                 recb[64:128, :])
            avt_of[(b, qc)] = avt

        COPY = mybir.ActivationFunctionType.Copy

        def part2_unit(b, qc, tt, pstag="op", split_dma=False):
            # chv0 copy on DVE, chv1 on ACT (Copy is in every act table, so
            # it never triggers a table re-load); tail units borrow the then
            # idle "sc" PSUM buffers to avoid WAR serialization
            avt = avt_of[(b, qc)]
            ost = sb.tile([128, 1024], BF16, tag="ost", bufs=6,
                          name=f"ost{b}_{qc}_{tt}")
            if pstag == "sc":
                big = ps.tile([128, 2, 512], F32, tag="sc", bufs=2,
                              name=f"opb{b}_{qc}_{tt}")
                ops = [big[:, 0, :], big[:, 1, :]]
            else:
                ops = [ps.tile([128, 512], F32, tag="op", bufs=2,
                               name=f"op{b}_{qc}_{tt}_{chv}")
                       for chv in range(2)]
            row0 = S * b + 512 * qc + 128 * tt
            for chv in range(2):
                nc.tensor.matmul(
                    ops[chv], lhsT=avt[:, 128 * tt:128 * (tt + 1)],
                    rhs=wo_t[:, 512 * chv:512 * (chv + 1)],
                    start=True, stop=True)
                dst = ost[:, 512 * chv:512 * (chv + 1)]
                if chv == 0:
                    nc.vector.tensor_copy(dst, ops[chv])
                else:
                    nc.scalar.activation(dst, ops[chv], COPY)
                if split_dma:
                    nc.sync.dma_start(
                        out_d[row0:row0 + 128, 512 * chv:512 * (chv + 1)],
                        dst)
            if not split_dma:
                nc.sync.dma_start(out_d[row0:row0 + 128, :], ost)

        # ---- attention ----
        rawf_of = {}

        def attention(b, p1_at, fillers):
            qt, ktz, vg = proj_out[b]
            rawf = [sb.tile([65, S], F32R, tag=f"rawf{h}", bufs=2,
                            name=f"rawf{b}_{h}") for h in range(2)]
            rawf_of[b] = rawf
            for qc in range(4):
                njt = 4 * qc + 4
                avps = [ps.tile([65, 512], F32, tag="av", bufs=2,
                                name=f"avps{b}_{qc}_{h}") for h in range(2)]
                fl = fillers[qc]
                nfl = len(fl)
                pend = []

                def do_av(j, et, avps=avps, qc=qc, njt=njt, vg=vg):
                    vs = max(0, 128 * (j - 4 * qc))
                    for h in range(2):
                        nc.tensor.matmul(
                            avps[h][0:65, vs:512],
                            lhsT=vg[:, j, 65 * h:65 * h + 65],
                            rhs=et[:, h, vs:512],
                            start=(j == 0), stop=(j == njt - 1),
                            skip_group_check=True)

                for j in range(njt):
                    vs = max(0, 128 * (j - 4 * qc))
                    sc = ps.tile([128, 2, 512], F32, tag="sc", bufs=2,
                                 name=f"sc{b}_{qc}_{j}")
                    for h in range(2):
                        nc.tensor.matmul(
                            sc[:, h, vs:512],
                            lhsT=ktz[h][:, 128 * j:128 * (j + 1)],
                            rhs=qt[:, 512 * qc + vs:512 * (qc + 1)],
                            start=True, stop=True)
                    et = sb.tile([128, 2, 512], BF16, tag="et", bufs=4,
                                 name=f"et{b}_{qc}_{j}")
                    nc.scalar.activation(et[:, :, vs:512], sc[:, :, vs:512],
                                         EXP, scale=SCALE)
                    if j >= 4 * qc:
                        for h in range(2):
                            nc.vector.tensor_mul(et[:, h, vs:vs + 128],
                                                 et[:, h, vs:vs + 128],
                                                 tri_t)
                    pend.append((j, et))
                    if len(pend) > 2:   # lag-2 software pipeline
                        do_av(*pend.pop(0))
                    if j == 1 and p1_at[qc] is not None:
                        p1_at[qc]()
                    # spread fillers over all but the last iteration: the
                    # rawf copies below gate the next qc's AV chain, so they
                    # must not queue behind the last slot's filler work
                    if j < njt - 1:
                        k0 = nfl * j // njt
                        k1 = nfl * (j + 1) // njt
                        for k in range(k0, k1):
                            fl[k]()
                for args in pend:
                    do_av(*args)
                for h in range(2):
                    nc.vector.tensor_copy(rawf[h][:, 512 * qc:512 * (qc + 1)],
                                          avps[h][0:65, :])
                for k in range(nfl * (njt - 1) // njt, nfl):
                    fl[k]()

        # ================= main flow =================
        # p-state warm-up: the PE needs ~3us of continuous work to reach
        # 2.4 GHz (else 1.2), and it would otherwise idle ~4.5us waiting for
        # the first DMAs. A stream of zero-input dummy matmuls (no data
        # deps) ramps the clock inside that dead window so the real
        # projections start at full speed.
        zt = sb.tile([128, 256], BF16, tag="zt", bufs=1, name="zt")
        nc.gpsimd.memset(zt, 0.0)
        dps = ps.tile([128, 256], F32, tag="op", bufs=2, name="dummy_ps")
        for _ in range(22):
            nc.tensor.matmul(dps, lhsT=zt[:, 0:128], rhs=zt,
                             start=True, stop=True, skip_group_check=True)

        units0 = proj_units(0)
        for u in units0:
            u()

        # b1 inputs: xt DMAs issue now (sync sequencer is idle during b0
        # attention; WAR on the xt pool delays them behind b0's V reads)
        for c in range(4):
            dma_xt(1, c)
        units1 = proj_units(1)

        def mk_p1(b, qc):
            return lambda: part1(b, qc, rawf_of[b])

        def mk_u(b, qc, tt):
            return lambda: part2_unit(b, qc, tt)

        # b0 attention: b1's projections + b0's own (per-qc-ready)
        # normalize/O-projection as PE fillers
        b0f0 = units1[0:4]                      # ones, Qc0, Kc0, Vc0a
        b0f1 = (units1[4:9] +                   # Vc0b, Qc1, Kc1, Vc1a/b
                [mk_u(0, 0, 0), mk_u(0, 0, 1), mk_u(0, 0, 2)])
        b0f2 = (units1[9:13] +                  # Qc2, Kc2, Vc2a/b
                [mk_u(0, 0, 3),
                 mk_u(0, 1, 0), mk_u(0, 1, 1), mk_u(0, 1, 2),
                 mk_u(0, 1, 3)])
        b0f3 = (units1[13:17] +                 # Qc3, Kc3, Vc3a/b
                [mk_u(0, 2, 0), mk_u(0, 2, 1), mk_u(0, 2, 2),
                 mk_u(0, 2, 3)])
        attention(0, p1_at=[None, mk_p1(0, 0), mk_p1(0, 1), mk_p1(0, 2)],
                  fillers=[b0f0, b0f1, b0f2, b0f3])

        # b1 attention: leftover b0 units + b1's own normalize/O
        f0 = [mk_u(0, 3, 0), mk_u(0, 3, 1), mk_u(0, 3, 2)]
        f1 = [mk_u(0, 3, 3),
              mk_u(1, 0, 0), mk_u(1, 0, 1), mk_u(1, 0, 2), mk_u(1, 0, 3)]
        f2 = [mk_u(1, 1, 0), mk_u(1, 1, 1)]
        f3 = [mk_u(1, 2, 0)]
        attention(1, p1_at=[mk_p1(0, 3), mk_p1(1, 0), mk_p1(1, 1),
                            mk_p1(1, 2)],
                  fillers=[f0, f1, f2, f3])

        # tail: the reserved units go FIRST so the in-order PE queue
        # executes them while part1(1,3)'s DVE chain resolves
        part2_unit(1, 1, 2, pstag="sc")
        part2_unit(1, 1, 3, pstag="op")
        part2_unit(1, 2, 1, pstag="sc")
        part2_unit(1, 2, 2, pstag="op")
        part2_unit(1, 2, 3, pstag="sc")
        part1(1, 3, rawf_of[1])
        for tt in range(4):
            part2_unit(1, 3, tt, pstag="op" if tt % 2 == 0 else "sc",
                       split_dma=True)
    nc.compile()
    return nc


def _get_built():
    global _BUILT
    if _BUILT is None:
        _BUILT = _build()
    return _BUILT


def _wslice(w, c):
    # [p, 8k x 128m]: w_sb[p, 128k+m] = w[128c+m, 128k+p]
    a = w[128 * c:128 * (c + 1)].reshape(128, 8, 128)
    return np.ascontiguousarray(a.transpose(2, 1, 0).reshape(128, D))


def _host_inputs(x, q_proj, k_proj, v_proj, o_proj):
    xr = x.reshape(NT, 8, 128).transpose(2, 1, 0)   # [p, k, t]
    xt = np.ascontiguousarray(xr).astype(BFNP)
    tri = np.triu(np.ones((128, 128), dtype=np.float32)).astype(BFNP)
    onesb = np.zeros((65, 256), dtype=np.float32)
    onesb[64, 0:64] = 1.0
    onesb[64, 192:256] = 1.0

    in_maps = []
    for c in range(NCORES):
        wq = _wslice(q_proj, c).astype(BFNP)
        wk = _wslice(k_proj, c).astype(BFNP)
        wvT = _wslice(v_proj, c).astype(BFNP)
        cb = np.ascontiguousarray(np.concatenate([wk, wvT, tri], axis=1))
        wo = np.ascontiguousarray(
            o_proj[:, 128 * c:128 * (c + 1)].T).astype(BFNP)
        in_maps.append(dict(xt=xt, wq=wq, cb=cb, wo=wo, onesb=onesb))
    return in_maps


def kernel(**inputs):
    x = np.asarray(inputs["x"], dtype=np.float32)
    q_proj = np.asarray(inputs["q_proj"], dtype=np.float32)
    k_proj = np.asarray(inputs["k_proj"], dtype=np.float32)
    v_proj = np.asarray(inputs["v_proj"], dtype=np.float32)
    o_proj = np.asarray(inputs["o_proj"], dtype=np.float32)

    in_maps = _host_inputs(x, q_proj, k_proj, v_proj, o_proj)
    nc = _get_built()
    global LAST_RESULTS
    LAST_RESULTS = run_bass_kernel_spmd(
        nc, in_maps, core_ids=list(range(NCORES)),
        trace=bool(os.environ.get("KERNEL_TRACE")))
    acc = np.asarray(LAST_RESULTS.results[0]["out"]).astype(np.float32)
    for c in range(1, NCORES):
        acc += np.asarray(LAST_RESULTS.results[c]["out"]).astype(np.float32)
    return acc.reshape(B, S, D)
